# revision 18
# baseline (speedup 1.0000x reference)
"""Trainium2 Bass kernel for nn_BDFM_46428596469849.

Per-batch math (B=8, C=256, H=W=128, HW=16384):
    m   = relu(m); z = (m > 0.3)
    er  = minpool4x4(z, SAME, border=1); di = maxpool4x4(z, SAME, border=0)
    fbu = [er, 1-di, di-er]                          # [3, HW]
    mid = fbu @ F^T                                  # [3, C]
    cf  = bn_f(Wf @ F);  mid1 = mid @ cf;  mid2 = mid^T @ mid1
    out = bn_o(W_out @ [F; mid2])

The chain collapses algebraically: with sf/bf (resp. so/bo) the BN scale/bias,
    g    = mid @ (diag(sf) Wf)            # [3, C]
    u    = mid @ bf                       # [3]
    A    = mid^T @ g                      # [C, C]
    v    = mid^T @ u                      # [C]
    Weff = W1 + W2 @ A                    # [C, C]   (W_out = [W1 | W2])
    out  = diag(so) @ Weff @ F + (so*(W2@v) + bo) 1^T
so each batch element needs only: the mid reduction (one pass over F with PE
transposes), tiny C x C algebra, and one C x C x HW matmul streamed over F.

Sharding: data-parallel, one batch element per NeuronCore (8 cores).

Implementation notes (v2):
  - F streams in as 16 x 1MB HWDGE DMAs ([128, 2048] fp32 tiles packing both
    128-channel chunks); GpSimd casts each piece to a resident bf16 copy.
    bf16 keeps the PE transpose at 1.0 cycles/row (vs 1.5 for f32r) and
    halves SBUF residency; measured end-to-end rel err ~4e-3 vs 2e-2 budget.
  - Small params (weights / BN vectors) load via HWDGE on the scalar queue;
    BN vectors land in one [1, 2048] staging row (one descriptor each) and
    are spread across partitions with tiny PE transposes, avoiding the
    4-byte-descriptor DMAs that polluted the load ramp.
  - mid accumulates in one [3, 512] PSUM tile (even h rows in cols 0:256,
    odd rows in 256:512) and is folded with one DVE add.
  - Final phase: [128, 1024] PSUM supertiles, 1024-wide matmuls, DVE/ACT
    alternating evictions into [128, 2048] staging, 16 x 1MB store DMAs.
"""

import os
import sys

for _p in ("/opt/trn_rl_repo", "/root/.axon_site/_ro/trn_rl_repo"):
    if os.path.isdir(_p) and _p not in sys.path:
        sys.path.insert(0, _p)

import numpy as np

import concourse.bass as bass
import concourse.mybir as mybir
import concourse.tile as tile
from concourse.bass_utils import run_bass_kernel_spmd
from concourse.masks import make_identity

dt = mybir.dt
AF = mybir.ActivationFunctionType
OP = mybir.AluOpType

B, C, H, W = 8, 256, 128, 128
HW = H * W
NCORES = 8
EPS = 1e-5
BF16 = dt.bfloat16

NPIECE = 8
PIECE = HW // NPIECE  # 2048 columns per piece, one tile per c-chunk


def _split_drain_waits(nc, max_waits=1):
    # Walrus codegen rejects instructions carrying more than a couple of
    # semaphore waits (CTRL drains and DMA descriptors in particular). Hoist
    # excess waits onto preceding NoOps on the same engine queue — the queue
    # executes in order, so the waits are satisfied before the instruction.
    for f in nc.m.functions:
        for bb in f.blocks:
            new_insts = []
            for inst in bb.instructions:
                si = inst.sync_info
                if si is not None and si.on_wait and len(si.on_wait) > max_waits:
                    waits = list(si.on_wait)
                    while len(waits) > max_waits:
                        chunk, waits = waits[:max_waits], waits[max_waits:]
                        pre = mybir.InstNoOp(
                            name=f"I-wsplit-{nc.next_id()}",
                            engine=inst.engine,
                            sync_info=mybir.SyncInfo(on_wait=chunk, on_update=[]),
                        )
                        nc.inst_map[pre.name] = pre
                        new_insts.append(pre)
                    inst.sync_info = mybir.SyncInfo(
                        on_wait=waits, on_update=list(si.on_update)
                    )
                new_insts.append(inst)
            bb.instructions[:] = new_insts


BN_KEYS = [
    f"bn_{pre}_{nm}" for pre in ("f", "o") for nm in ("gamma", "beta", "mean", "var")
]


def build_nc():
    from contextlib import ExitStack

    nc = bass.Bass("TRN2", target_bir_lowering=False)

    feat = nc.declare_dram_parameter("feature", [C, HW], dt.float32, isOutput=False)
    m_in = nc.declare_dram_parameter("m", [H, W], dt.float32, isOutput=False)
    wfeat = nc.declare_dram_parameter("w_feat", [C, C], dt.float32, isOutput=False)
    wout = nc.declare_dram_parameter("w_out", [C, 2 * C], dt.float32, isOutput=False)
    bnp = {k: nc.declare_dram_parameter(k, [C], dt.float32, isOutput=False)
           for k in BN_KEYS}
    out_d = nc.declare_dram_parameter("out", [C, HW], BF16, isOutput=True)

    with tile.TileContext(nc) as tc, ExitStack() as ctx:
        const = ctx.enter_context(tc.tile_pool(name="const", bufs=1))
        ident = const.tile([128, 128], dt.float32, name="ident")
        make_identity(nc, ident)
        ident_b = const.tile([128, 128], BF16, name="ident_b")
        nc.vector.tensor_copy(ident_b, ident)
        eps_t = const.tile([128, 1], dt.float32, name="eps_t")
        nc.vector.memset(eps_t, EPS)

        # ---- m first on the sync queue (morphology is on the critical path),
        # then the F pieces right behind it; weights/BN go on the scalar
        # HWDGE queue so the sync queue never stalls.
        m_sb = const.tile([128, 128], dt.float32, name="m_sb")
        nc.sync.dma_start(out=m_sb, in_=m_in[:, :])

        # ---- resident feature: SWDGE cast-during-DMA straight to bf16 ----
        # (fp32 in HBM -> bf16 in SBUF; the cast happens in the SDMA datapath,
        # costing no engine time.) Pieces are per-c-chunk [128, 2048] slabs so
        # every partition row is ONE contiguous 8KB descriptor: 128
        # descriptors per DMA keeps the whole load inside the SWDGE
        # descriptor ring - 256-descriptor DMAs made Q7 backpressure after
        # ~8 pieces and stretched the load tail by ~8us.
        fpool = ctx.enter_context(tc.tile_pool(name="fpool", bufs=1))
        F_t = [
            [
                fpool.tile([128, PIECE], BF16, name=f"F{cc}_{j}", tag=f"F{cc}_{j}")
                for j in range(NPIECE)
            ]
            for cc in range(2)
        ]

        def emit_load(j):
            for cc in range(2):
                nc.gpsimd.dma_start(
                    out=F_t[cc][j],
                    in_=feat[
                        cc * 128 : (cc + 1) * 128, j * PIECE : (j + 1) * PIECE
                    ],
                )

        def f_slice(cc, col0, width):
            j = col0 // PIECE
            off = col0 % PIECE
            assert off + width <= PIECE
            return F_t[cc][j][:, off : off + width]

        # ---- small inputs on the scalar HWDGE queue ----
        wf = []
        wo = []
        for oc in range(2):
            t = const.tile([128, C], dt.float32, name=f"wf{oc}", tag=f"wf{oc}")
            nc.scalar.dma_start(out=t, in_=wfeat[oc * 128 : (oc + 1) * 128, :])
            wf.append(t)
            t2 = const.tile([128, 2 * C], dt.float32, name=f"wo{oc}", tag=f"wo{oc}")
            nc.scalar.dma_start(out=t2, in_=wout[oc * 128 : (oc + 1) * 128, :])
            wo.append(t2)

        # BN vectors: one contiguous [1, 256] descriptor each into a single
        # staging row; spread to [128, 16] via 16 tiny PE transposes.
        bn_stage = const.tile([1, 2048], dt.float32, name="bn_stage")
        for pi, key in enumerate(BN_KEYS):
            nc.scalar.dma_start(
                out=bn_stage[0:1, pi * 256 : (pi + 1) * 256],
                in_=bnp[key][:].rearrange("(o c) -> o c", o=1),
            )
        bn_sb = const.tile([128, 16], dt.float32, name="bn_sb")
        with tc.tile_pool(name="bn_ps", bufs=1, space="PSUM") as bn_ps_pool:
            bn_ps = bn_ps_pool.tile([128, 16], dt.float32, name="bn_ps")
            for j in range(16):
                nc.tensor.transpose(
                    bn_ps[:, j : j + 1],
                    bn_stage[0:1, j * 128 : (j + 1) * 128],
                    ident[0:1, 0:1],
                )
            nc.vector.tensor_copy(bn_sb, bn_ps)

        def bn_col(key, cc):
            pi = BN_KEYS.index(key)
            return bn_sb[:, 2 * pi + cc : 2 * pi + cc + 1]

        def bn_cols(key):
            pi = BN_KEYS.index(key)
            return bn_sb[:, 2 * pi : 2 * pi + 2]

        # ---- morphology: separable 4x4 window (offsets -1..+2), both passes
        # along the free dim with a PE transpose in between; border = the
        # reduction identity (matches reduce_window SAME + init value) ----
        mor = ctx.enter_context(tc.tile_pool(name="mor", bufs=1))

        def pool1d_free(eng, src, op, border, label):
            padd = mor.tile([128, 131], dt.float32, name=f"pad_{label}", tag=f"pad_{label}")
            eng.memset(padd, border)
            eng.tensor_copy(padd[:, 1:129], src)
            a = mor.tile([128, 130], dt.float32, name=f"a_{label}", tag=f"a_{label}")
            eng.tensor_tensor(a, padd[:, 0:130], padd[:, 1:131], op)
            r = mor.tile([128, 128], dt.float32, name=f"r_{label}", tag=f"r_{label}")
            eng.tensor_tensor(r, a[:, 0:128], a[:, 2:130], op)
            return r

        z = mor.tile([128, 128], dt.float32, name="z")
        nc.vector.tensor_scalar(out=z, in0=m_sb, scalar1=0.3, scalar2=None, op0=OP.is_gt)
        erw = pool1d_free(nc.vector, z, OP.min, 1.0, "er1")  # [h, w] pooled over w
        diw = pool1d_free(nc.vector, z, OP.max, 0.0, "di1")
        with tc.tile_pool(name="mor_ps", bufs=1, space="PSUM") as mor_ps:
            er_ps = mor_ps.tile([128, 128], dt.float32, name="er_ps", tag="er_ps")
            nc.tensor.transpose(er_ps, erw, ident)
            erwT = mor.tile([128, 128], dt.float32, name="erwT")
            nc.vector.tensor_copy(erwT, er_ps)
            di_ps = mor_ps.tile([128, 128], dt.float32, name="di_ps", tag="di_ps")
            nc.tensor.transpose(di_ps, diw, ident)
            diwT = mor.tile([128, 128], dt.float32, name="diwT")
            nc.vector.tensor_copy(diwT, di_ps)
        erT = pool1d_free(nc.vector, erwT, OP.min, 1.0, "er2")  # [w, h] pooled over h
        diT = pool1d_free(nc.vector, diwT, OP.max, 0.0, "di2")

        # fbuT[w, h, k] in bf16 (mask values are exact); [:, h, :] slices are
        # contiguous [128, 3] APs for the mid matmul's stationary operand.
        fbuT = mor.tile([128, 128, 3], BF16, name="fbuT")
        nc.vector.tensor_copy(fbuT[:, :, 0], erT)
        nc.vector.tensor_scalar(
            out=fbuT[:, :, 1], in0=diT, scalar1=-1.0, scalar2=1.0, op0=OP.mult, op1=OP.add
        )
        nc.vector.tensor_tensor(fbuT[:, :, 2], diT, erT, OP.subtract)

        # ---- BN scale/bias: s = gamma*rsqrt(var+eps), b = beta - mean*s ----
        setup = ctx.enter_context(tc.tile_pool(name="setup", bufs=1))

        def bn_prep(pre):
            s = setup.tile([128, 2], dt.float32, name=f"s_{pre}", tag=f"s_{pre}")
            b = setup.tile([128, 2], dt.float32, name=f"b_{pre}", tag=f"b_{pre}")
            tmp = setup.tile([128, 2], dt.float32, name=f"tmp_{pre}", tag=f"tmp_{pre}")
            nc.scalar.activation(
                out=tmp, in_=bn_cols(f"bn_{pre}_var"), func=AF.Sqrt, bias=eps_t, scale=1.0
            )
            nc.vector.reciprocal(out=tmp, in_=tmp)
            nc.vector.tensor_mul(s, bn_cols(f"bn_{pre}_gamma"), tmp)
            nc.vector.tensor_mul(tmp, bn_cols(f"bn_{pre}_mean"), s)
            nc.vector.tensor_sub(b, bn_cols(f"bn_{pre}_beta"), tmp)
            return s, b

        sf, bf = bn_prep("f")
        so, bo = bn_prep("o")

        alg = ctx.enter_context(tc.tile_pool(name="alg", bufs=1))

        def emit_rhs_g():
            # rhs = [diag(sf) Wf | bf] per o-chunk (feeds g_ext = mid @ rhs)
            for cc in range(2):
                r = alg.tile([128, C + 1], BF16, name=f"rhs_g{cc}", tag=f"rhs_g{cc}")
                nc.vector.tensor_scalar(
                    out=r[:, 0:C], in0=wf[cc], scalar1=sf[:, cc : cc + 1],
                    scalar2=None, op0=OP.mult,
                )
                nc.vector.tensor_copy(r[:, C : C + 1], bf[:, cc : cc + 1])
                rhs_g.append(r)

        def emit_w2t(w2t_ps_pool):
            # W2T[j][128, 256] via bf16 transpose of W2 blocks
            for jc in range(2):
                W2T_ps = w2t_ps_pool.tile([128, C], BF16, name="W2T_ps", tag="W2T_ps")
                for oc in range(2):
                    nc.tensor.transpose(
                        W2T_ps[:, oc * 128 : (oc + 1) * 128],
                        wo_b[oc][:, C + jc * 128 : C + (jc + 1) * 128],
                        ident_b,
                    )
                t = alg.tile([128, C], BF16, name=f"W2T{jc}", tag=f"W2T{jc}")
                nc.vector.tensor_copy(t, W2T_ps)
                W2T_sb.append(t)

        rhs_g = []
        W2T_sb = []
        wo_b = []

        def emit_wo_b():
            for oc in range(2):
                t = alg.tile([128, 2 * C], BF16, name=f"wo_b{oc}", tag=f"wo_b{oc}")
                nc.vector.tensor_copy(t, wo[oc])
                wo_b.append(t)

        # ---- mid = fbu @ F^T via per-h PE transposes, accumulated in PSUM.
        # Per 2-row group hp: 4 bf16 transposes of [128,128] chunks into one
        # [128, 512] PSUM tile (order h0cc0|h0cc1|h1cc0|h1cc1), one eviction
        # to bf16 SBUF (engines alternate), then 2 accumulating matmuls:
        # even h rows into mid_ps cols 0:256, odd rows into 256:512.
        mid_sb = alg.tile([3, C], BF16, name="mid_sb")
        with tc.tile_pool(name="midps", bufs=1, space="PSUM") as midps:
            mid_psA = midps.tile([3, C], dt.float32, name="mid_psA")
            mid_psB = midps.tile([3, C], dt.float32, name="mid_psB")
            with tc.tile_pool(name="tr_ps", bufs=4, space="PSUM") as tr_ps_pool, \
                 tc.tile_pool(name="f1T_pool", bufs=16) as f1T_pool, \
                 tc.tile_pool(name="w2t_ps_pool", bufs=2, space="PSUM") as w2t_ps_pool:
                for i in range(NPIECE):
                    emit_load(i)
                GPP = 64 // NPIECE  # hp-groups per piece
                # mid matmuls trail the transposes by one full piece (4
                # hp-groups) and are batched so the PE pays the weight-port
                # transpose<->matmul switch penalty twice per piece, not
                # twice per hp-group

                def emit_mid(hp, f1T):
                    for q2 in range(2):
                        h = 2 * hp + q2
                        nc.tensor.matmul(
                            mid_psA[:, :] if q2 == 0 else mid_psB[:, :],
                            lhsT=fbuT[:, h, :],
                            rhs=f1T[:, q2 * 256 : (q2 + 1) * 256],
                            start=(hp == 0),
                            stop=(hp == 63),
                        )

                pending = []
                for i in range(NPIECE):
                    if i == 2:
                        emit_wo_b()
                        emit_rhs_g()
                        emit_w2t(w2t_ps_pool)
                    for g in range(GPP):
                        hp = GPP * i + g
                        tps = tr_ps_pool.tile([128, 512], BF16, name="tps")
                        for q in range(4):
                            h = 2 * hp + q // 2
                            cc = q % 2
                            nc.tensor.transpose(
                                tps[:, q * 128 : (q + 1) * 128],
                                f_slice(cc, h * 128, 128),
                                ident_b,
                            )
                        # split the eviction across both engines; the LAG
                        # must cover the slower (ACT) half's start latency
                        f1T = f1T_pool.tile([128, 512], BF16, name="f1T")
                        nc.vector.tensor_copy(f1T[:, 0:256], tps[:, 0:256])
                        nc.scalar.copy(f1T[:, 256:512], tps[:, 256:512])
                        pending.append((hp, f1T))
                    if i >= 1:
                        for hp_f in pending[:GPP]:
                            emit_mid(*hp_f)
                        del pending[:GPP]
                for hp_f in pending:
                    emit_mid(*hp_f)
            # fold even/odd accumulators (only one PSUM read per op)
            nc.vector.tensor_copy(mid_sb, mid_psA)
            nc.vector.tensor_tensor(mid_sb, mid_sb, mid_psB, OP.add)

        # ---- tiny algebra: g_ext, A_ext, W2T, WeffT, beff (plain fp32) ----
        WeffT_b = []
        with tc.tile_pool(name="alg_ps", bufs=1, space="PSUM") as alg_ps:
            midT_sb = alg.tile([128, 6], BF16, name="midT_sb")
            for cc in range(2):
                mT2 = alg_ps.tile([128, 3], BF16, name="mT2", tag="mT2")
                nc.tensor.transpose(
                    mT2, mid_sb[:, cc * 128 : (cc + 1) * 128], ident_b[0:3, 0:3]
                )
                nc.vector.tensor_copy(midT_sb[:, cc * 3 : (cc + 1) * 3], mT2)

            gext_ps = alg_ps.tile([3, C + 1], dt.float32, name="gext_ps", tag="gext_ps")
            for cc in range(2):
                nc.tensor.matmul(
                    gext_ps,
                    lhsT=midT_sb[:, cc * 3 : (cc + 1) * 3],
                    rhs=rhs_g[cc],
                    start=(cc == 0),
                    stop=(cc == 1),
                )
            gext_sb = alg.tile([3, C + 1], BF16, name="gext_sb")
            nc.vector.tensor_copy(gext_sb, gext_ps)

            # A_ext = mid^T @ g_ext -> [C, 257]; col 256 is v = mid^T u
            A_sb = []
            for cc in range(2):
                A_ps = alg_ps.tile([128, C + 1], dt.float32, name="A_ps", tag="A_ps")
                nc.tensor.matmul(
                    A_ps, lhsT=mid_sb[:, cc * 128 : (cc + 1) * 128], rhs=gext_sb,
                    start=True, stop=True,
                )
                t = alg.tile([128, C + 1], BF16, name=f"A{cc}", tag=f"A{cc}")
                nc.vector.tensor_copy(t, A_ps)
                A_sb.append(t)

            # WeffT = W1^T + A^T @ W2T  (W1^T added via identity matmuls)
            for cc in range(2):
                Wt_ps = alg_ps.tile([128, C], dt.float32, name="Wt_ps", tag="Wt_ps")
                for j in range(2):
                    nc.tensor.matmul(
                        Wt_ps,
                        lhsT=A_sb[j][:, cc * 128 : (cc + 1) * 128],
                        rhs=W2T_sb[j],
                        start=(j == 0),
                        stop=False,
                    )
                for oc in range(2):
                    nc.tensor.matmul(
                        Wt_ps[:, oc * 128 : (oc + 1) * 128],
                        lhsT=wo_b[oc][:, cc * 128 : (cc + 1) * 128],
                        rhs=ident_b,
                        start=False,
                        stop=(oc == 1),
                    )
                t = alg.tile([128, C], BF16, name=f"WeffTb{cc}", tag=f"WeffTb{cc}")
                nc.vector.tensor_copy(t, Wt_ps)
                WeffT_b.append(t)

            # beff = so * (W2 @ v) + bo
            beff = alg.tile([128, 2], dt.float32, name="beff")
            for oc in range(2):
                wv_ps = alg_ps.tile([128, 1], dt.float32, name="wv_ps", tag="wv_ps")
                for j in range(2):
                    nc.tensor.matmul(
                        wv_ps,
                        lhsT=W2T_sb[j][:, oc * 128 : (oc + 1) * 128],
                        rhs=A_sb[j][:, C : C + 1],
                        start=(j == 0),
                        stop=(j == 1),
                    )
                nc.vector.tensor_scalar(
                    out=beff[:, oc : oc + 1], in0=wv_ps,
                    scalar1=so[:, oc : oc + 1], scalar2=bo[:, oc : oc + 1],
                    op0=OP.mult, op1=OP.add,
                )

        # ---- final: out = so * (Weff @ F) + beff, streamed over n ----
        # Per (o-chunk, piece): two [128, 1024] PSUM supertiles (four
        # 512-wide accumulating matmuls each), evicted alternately by
        # DVE/ACT into halves of a [128, 2048] bf16 staging tile, then one
        # store DMA on the sync queue.
        with tc.tile_pool(name="fin_ps", bufs=4, space="PSUM") as fin_ps, \
             tc.tile_pool(name="osb", bufs=4) as osb_pool:
            for oc in range(2):
                for j in range(NPIECE):
                    ot = osb_pool.tile([128, PIECE], BF16, name="ot")
                    for half in range(2):
                        ps2 = fin_ps.tile([128, PIECE // 2], dt.float32, name="ps2")
                        for cc in range(2):
                            for t in range(2):
                                nc.tensor.matmul(
                                    ps2[:, t * 512 : (t + 1) * 512],
                                    lhsT=WeffT_b[cc][:, oc * 128 : (oc + 1) * 128],
                                    rhs=F_t[cc][j][
                                        :,
                                        half * 1024 + t * 512 : half * 1024 + (t + 1) * 512,
                                    ],
                                    start=(cc == 0),
                                    stop=(cc == 1),
                                )
                        if half == 0:
                            nc.vector.tensor_scalar(
                                out=ot[:, 0:1024],
                                in0=ps2, scalar1=so[:, oc : oc + 1],
                                scalar2=beff[:, oc : oc + 1], op0=OP.mult, op1=OP.add,
                            )
                        else:
                            nc.scalar.activation(
                                out=ot[:, 1024:2048],
                                in_=ps2, func=AF.Identity,
                                bias=beff[:, oc : oc + 1], scale=so[:, oc : oc + 1],
                            )
                    nc.sync.dma_start(
                        out=out_d[
                            oc * 128 : (oc + 1) * 128, j * PIECE : (j + 1) * PIECE
                        ],
                        in_=ot,
                    )

    _split_drain_waits(nc)
    return nc


_NC_CACHE = None


def _get_nc():
    global _NC_CACHE
    if _NC_CACHE is None:
        _NC_CACHE = build_nc()
    return _NC_CACHE


def kernel(**inputs):
    feature = np.asarray(inputs["feature"], dtype=np.float32)
    m = np.asarray(inputs["m"], dtype=np.float32)
    shared = {}
    shared["w_feat"] = np.asarray(inputs["w_feat"], dtype=np.float32)
    shared["w_out"] = np.asarray(inputs["w_out"], dtype=np.float32)
    for pre in ("f", "o"):
        for nm in ("gamma", "beta", "mean", "var"):
            key = f"bn_{pre}_{nm}"
            shared[key] = np.asarray(inputs[key], dtype=np.float32)

    nc = _get_nc()
    in_maps = []
    for i in range(NCORES):
        im = dict(shared)
        im["feature"] = np.ascontiguousarray(feature[i].reshape(C, HW))
        im["m"] = np.ascontiguousarray(m[i].reshape(H, W))
        in_maps.append(im)

    res = run_bass_kernel_spmd(nc, in_maps, core_ids=list(range(NCORES)))
    out = np.stack(
        [
            np.asarray(res.results[i]["out"]).astype(np.float32).reshape(C, H, W)
            for i in range(NCORES)
        ]
    )
    return out
